# revision 78
# baseline (speedup 1.0000x reference)
"""Multi-head causal linear attention (B=1, N=2048, D=1024, H=16) on 8 trn2 cores.

Math: reference computes, per head (e=64):
    q = softmax(q_raw, -1) * e**-0.5 ;  k = exp(k_raw)
    out_n = (q_n . KV_n) / (q_n . (kcum_n + EPS)),  KV_n = sum_{j<=n} k_j v_j^T
Because both numerator and denominator are linear in q_n, the softmax
normalization and the e**-0.5 scale cancel exactly; only u = exp(q_raw)
matters.  The EPS term contributes <1e-6 relative and is dropped.  The
v-bias contribution factors out:  out += b_v  (sum_j s_nj / denom ~= 1).

Per-core work (head-parallel, 2 heads/core):
    qvk^T = W_c^T @ x  computed as matmul(lhsT=W_block, rhs=x^T) on PE,
    x^T is pre-transposed on the host so no on-chip transpose is needed.
    Chunked causal linear attention (chunk=128) with the classic
    intra (masked QK^T V) + inter (running KV state) recurrence.
"""

import os
from contextlib import ExitStack

import numpy as np

import concourse.bass as bass
import concourse.mybir as mybir
import concourse.tile as tile
from concourse import bacc
from concourse._compat import with_exitstack
from concourse.bass import ts

FP32 = mybir.dt.float32
F32R = mybir.dt.float32r
BF16 = mybir.dt.bfloat16

B, N, D, H = 1, 2048, 1024, 16
E = D // H          # 64 head dim
NCORES = 8
HPC = H // NCORES   # 2 heads per core
F = 3 * HPC * E     # 384 per-core projected features (q-pair | k-pair | v-pair)
KT = D // 128       # 8 contraction tiles
TT = 512            # token tile (projection granularity)
NTT = N // TT       # 4
C = 128             # chunk (tokens) for the causal recurrence
CPT = TT // C       # 4 chunks per token tile
NC = N // C         # 16 chunks total

Exp = mybir.ActivationFunctionType.Exp
MULT = mybir.AluOpType.mult
ADD = mybir.AluOpType.add


@with_exitstack
def _emit(ctx: ExitStack, tc, io):
    nc = tc.nc
    mega_d, cf_d, xt1_d, xt23_d, out_d = io

    const = ctx.enter_context(tc.tile_pool(name="const", bufs=1))
    chain = ctx.enter_context(tc.tile_pool(name="chain", bufs=2))
    smtp = ctx.enter_context(tc.tile_pool(name="smtp", bufs=2))
    small = ctx.enter_context(tc.tile_pool(name="small", bufs=3))
    outp = ctx.enter_context(tc.tile_pool(name="outp", bufs=3))
    pproj = ctx.enter_context(tc.tile_pool(name="pproj", bufs=2, space="PSUM"))
    ps_s = ctx.enter_context(tc.tile_pool(name="ps_s", bufs=2, space="PSUM"))
    ps_tr = ctx.enter_context(tc.tile_pool(name="ps_tr", bufs=1, space="PSUM"))
    ps_out = ctx.enter_context(tc.tile_pool(name="ps_out", bufs=2, space="PSUM"))
    ps_st = ctx.enter_context(tc.tile_pool(name="ps_st", bufs=1, space="PSUM"))

    # ---- persistent SBUF (packed: one big DMA gates the first matmul) ----
    # mega: [ W (k f) 3072 | ident 128 | xt(tt=0) (k t) 4096 ]
    mega_sb = const.tile([128, KT * F + 128 + KT * TT], BF16)
    cf_sb = const.tile([128, 2 + HPC * E + TT], FP32)  # [bq|bk|bv|mask]
    xtr_sb = const.tile([128, (NTT - 1) * KT * TT], BF16)  # xt tt=1..3, (tt k t)

    id_sb = mega_sb[:, KT * F : KT * F + 128]
    bq_sb = cf_sb[:, 0:1]
    bk_sb = cf_sb[:, 1:2]
    bv_sb = cf_sb[:, 2 : 2 + HPC * E]
    mask_sb = cf_sb[:, 2 + HPC * E :]

    def w_ap(k, f):
        return mega_sb[:, k * F + f * 128 : k * F + (f + 1) * 128]

    def xt_ap(tt, k):
        if tt == 0:
            base = KT * F + 128 + k * TT
            return mega_sb[:, base : base + TT]
        base = (tt - 1) * KT * TT + k * TT
        return xtr_sb[:, base : base + TT]

    # PE warm-up bridge: junk matmuls on zeros keep the HAM activity window
    # busy while inputs stream in, so real matmuls start at 2.4 GHz.
    scratch = const.tile([128, TT], BF16)
    nc.gpsimd.memset(scratch[:, :], 0.0)
    junk_ps = pproj.tile([128, TT], FP32, tag="proj")
    for j in range(14):
        nc.tensor.matmul(
            junk_ps[:, :],
            lhsT=scratch[:, 0:128],
            rhs=scratch[:, :],
            start=True,
            stop=True,
        )

    # W + ident first, then xt(tt=0) in two halves so the first token
    # tile's projection k-loop can start as soon as the first half lands.
    WX0 = KT * F + 128
    MID = WX0 + 4 * TT
    nc.sync.dma_start(mega_sb[:, 0:WX0], mega_d[:, 0:WX0])
    nc.sync.dma_start(mega_sb[:, WX0:MID], mega_d[:, WX0:MID])
    nc.sync.dma_start(mega_sb[:, MID:], mega_d[:, MID:])
    nc.sync.dma_start(cf_sb[:, :], cf_d[:, :])
    nc.sync.dma_start(xtr_sb[:, 0 : KT * TT], xt1_d[:, :])
    nc.sync.dma_start(xtr_sb[:, KT * TT :], xt23_d[:, :])

    # running KV state, head h on partitions [64h:64h+64]; col 64 = k-sum
    kv_prev = None   # bf16 copy of the state (matmul operand)
    kv_f32 = None    # fp32 master accumulator
    pending = None
    dma_flip = [0]

    def finalize(out_ps, osb, ftt, fcc):
        fc = ftt * CPT + fcc
        last_chunk = ftt == NTT - 1 and fcc == CPT - 1
        rec = small.tile([128, HPC], FP32, tag="rec", name=f"rec{fc}")
        nc.vector.reciprocal(rec[:, :], out_ps[:, :, E])
        for h in range(HPC):
            nc.vector.scalar_tensor_tensor(
                osb[:, fcc, ts(h, E)],
                in0=out_ps[:, h, 0:E],
                scalar=rec[:, h : h + 1],
                in1=bv_sb[:, ts(h, E)],
                op0=MULT,
                op1=ADD,
            )
            if last_chunk:
                # very last chunk: ship each head half right after its own
                # stt, on separate trigger engines, so the end-of-kernel
                # drain waits on a smaller, earlier transfer
                eng2 = nc.sync if h == 0 else nc.gpsimd
                eng2.dma_start(
                    out_d[ts(fc, C), ts(h, E)], osb[:, fcc, ts(h, E)]
                )
        if last_chunk:
            return
        eng = nc.gpsimd if dma_flip[0] % 2 else nc.sync
        if ftt == NTT - 1:
            # last token tile: ship each chunk as soon as it's done, on
            # alternating trigger engines, to shorten the kernel tail
            dma_flip[0] += 1
            eng.dma_start(out_d[ts(fc, C), :], osb[:, fcc, :])
        elif fcc == CPT - 1:
            dma_flip[0] += 1
            eng.dma_start(
                out_d[ts(ftt, TT), :].rearrange("(cc p) f -> p cc f", p=128),
                osb[:, :, :],
            )

    st = [dict() for _ in range(NTT)]

    def emit_proj_f(tt, f):
        # projection: qvk^T[f, t] = sum_d W[d, f] * xT[d, t]
        s = st[tt]
        pp = pproj.tile([128, TT], FP32, tag="proj", name=f"pp{tt}_{f}")
        for k in range(KT):
            nc.tensor.matmul(
                pp[:, :],
                lhsT=w_ap(k, f),
                rhs=xt_ap(tt, k),
                start=(k == 0),
                stop=(k == KT - 1),
            )
        if f == 0:
            s["UT"] = UT = chain.tile([128, TT], BF16, tag="UT", name=f"UT{tt}")
            nc.scalar.activation(UT[:, :], pp[:, :], Exp, bias=bq_sb[:, 0:1])
        elif f == 1:
            s["EkT"] = EkT = chain.tile([128, TT], BF16, tag="EkT", name=f"EkT{tt}")
            nc.scalar.activation(EkT[:, :], pp[:, :], Exp, bias=bk_sb[:, 0:1])
        else:
            s["VT"] = VT = chain.tile([128, TT], BF16, tag="VT", name=f"VT{tt}")
            nc.scalar.copy(VT[:, :], pp[:, :])

    def emit_prepare(tt):
        # token-layout Ek / V (PE transpose + ACT copy), chunk scores, masks
        s = st[tt]
        UT, EkT, VT = s["UT"], s["EkT"], s["VT"]
        s["ek_toks"], s["v_augs"] = [], []
        for cc in range(CPT):
            trp = ps_tr.tile([128, 256], BF16, tag="tr", name=f"trp{tt}_{cc}")
            nc.tensor.transpose(trp[:, 0:128], EkT[:, ts(cc, C)], id_sb[:, :])
            nc.tensor.transpose(trp[:, 128:256], VT[:, ts(cc, C)], id_sb[:, :])
            ek_tok = small.tile(
                [128, 128], BF16, tag="ektok", bufs=9, name=f"ektok{tt}_{cc}"
            )
            nc.scalar.copy(ek_tok[:, :], trp[:, 0:128])
            v_aug = small.tile(
                [128, HPC, E + 1], BF16, tag="vaug", bufs=9, name=f"vaug{tt}_{cc}"
            )
            nc.scalar.copy(
                v_aug[:, :, 0:E],
                trp[:, 128:256].rearrange("p (g e) -> p g e", g=HPC),
            )
            nc.gpsimd.memset(v_aug[:, :, E : E + 1], 1.0)
            s["ek_toks"].append(ek_tok)
            s["v_augs"].append(v_aug)
        # S^T[j, i] = sum_d Ek[j,d] U[i,d]
        sps = [
            ps_s.tile([128, TT], FP32, tag="s", name=f"sp{tt}_{h}")
            for h in range(HPC)
        ]
        for cc in range(CPT):
            for h in range(HPC):
                nc.tensor.matmul(
                    sps[h][:, ts(cc, C)],
                    lhsT=EkT[ts(h, E), ts(cc, C)],
                    rhs=UT[ts(h, E), ts(cc, C)],
                    start=True,
                    stop=True,
                    tile_position=(E * h, 0),
                )
        s["smt"] = []
        for h in range(HPC):
            sm = smtp.tile([128, TT], BF16, tag=f"smt{h}", name=f"smt{tt}_{h}")
            nc.vector.tensor_mul(sm[:, :], sps[h][:, :], mask_sb[:, :])
            s["smt"].append(sm)

    def emit_chain_chunk(tt, cc):
        nonlocal kv_prev, kv_f32, osb
        s = st[tt]
        UT, smt = s["UT"], s["smt"]
        c = tt * CPT + cc
        ek_tok = s["ek_toks"][cc]
        v_aug = s["v_augs"][cc]
        out_ps = ps_out.tile([128, HPC, E + 1], FP32, tag="out", name=f"ops{c}")
        delta_ps = ps_st.tile([128, E + 1], FP32, tag="delta", name=f"dps{c}")
        for h in range(HPC):
            nc.tensor.matmul(
                out_ps[:, h, :],
                lhsT=smt[h][:, ts(cc, C)],
                rhs=v_aug[:, h, :],
                start=True,
                stop=(c == 0),
            )
            if c > 0:
                nc.tensor.matmul(
                    out_ps[:, h, :],
                    lhsT=UT[ts(h, E), ts(cc, C)],
                    rhs=kv_prev[ts(h, E), :],
                    start=False,
                    stop=True,
                    tile_position=(E * h, 0),
                )
        for h in range(HPC):
            nc.tensor.matmul(
                delta_ps[ts(h, E), :],
                lhsT=ek_tok[:, ts(h, E)],
                rhs=v_aug[:, h, :],
                start=True,
                stop=True,
                tile_position=(0, E * h),
            )
        if c < NC - 1:
            kv_bf = small.tile([128, E + 1], BF16, tag="kv", name=f"kvb{c}")
            kv_new = small.tile([128, E + 1], FP32, tag="kvm", name=f"kvm{c}")
            if c == 0:
                nc.vector.tensor_copy(kv_bf[:, :], delta_ps[:, :])
                nc.vector.tensor_copy(kv_new[:, :], delta_ps[:, :])
            else:
                nc.vector.tensor_add(kv_bf[:, :], delta_ps[:, :], kv_f32[:, :])
                nc.vector.tensor_add(kv_new[:, :], delta_ps[:, :], kv_f32[:, :])
            kv_prev, kv_f32 = kv_bf, kv_new

        if cc == 0:
            osb = outp.tile([128, CPT, HPC * E], FP32, tag="osb", name=f"osb{tt}")
        finalize(out_ps, osb, tt, cc)

    # ---- software-pipelined emission: the next tile's projection / prep
    # instructions are woven between the chain chunks so the PE's in-order
    # stream always has independent work queued behind each cross-engine
    # dependency of the sequential state chain.
    osb = None
    for f in range(3):
        emit_proj_f(0, f)
    emit_prepare(0)
    for tt in range(NTT):
        if tt < NTT - 1:
            nxt = tt + 1
            slices = [lambda f=f: emit_proj_f(nxt, f) for f in range(3)] + [
                lambda: emit_prepare(nxt)
            ]
        else:
            slices = [None] * CPT
        for cc in range(CPT):
            emit_chain_chunk(tt, cc)
            if slices[cc] is not None:
                slices[cc]()


def build_nc():
    nc = bacc.Bacc(
        "TRN2",
        target_bir_lowering=False,
        debug=False,
        enable_asserts=False,
        num_devices=NCORES,
    )
    mega_d = nc.dram_tensor(
        "mega", [128, KT * F + 128 + KT * TT], BF16, kind="ExternalInput"
    ).ap()
    cf_d = nc.dram_tensor(
        "cf", [128, 2 + HPC * E + TT], FP32, kind="ExternalInput"
    ).ap()
    xt1_d = nc.dram_tensor("xt1", [128, KT * TT], BF16, kind="ExternalInput").ap()
    xt23_d = nc.dram_tensor(
        "xt23", [128, 2 * KT * TT], BF16, kind="ExternalInput"
    ).ap()
    out_d = nc.dram_tensor("out", [N, HPC * E], FP32, kind="ExternalOutput").ap()
    io = (mega_d, cf_d, xt1_d, xt23_d, out_d)
    with tile.TileContext(nc) as tc:
        _emit(tc, io)
    nc.compile()
    return nc


def host_inputs(x, W_qvk, b_qvk):
    """Full inputs -> per-core in_maps (host-side shard + transpose)."""
    import ml_dtypes

    x = np.asarray(x, dtype=np.float32).reshape(N, D)
    W = np.asarray(W_qvk, dtype=np.float32)
    b = np.asarray(b_qvk, dtype=np.float32)
    xt = x.T.astype(ml_dtypes.bfloat16)  # (D, N)

    def pack(a):  # (D, M) -> (128, KT*M), partition-contiguous
        kt, m = a.shape[0] // 128, a.shape[1]
        return np.ascontiguousarray(
            a.reshape(kt, 128, m).transpose(1, 0, 2).reshape(128, kt * m)
        )

    xtp = [pack(xt[:, tt * TT : (tt + 1) * TT]) for tt in range(NTT)]
    xt1 = xtp[1]
    xt23 = np.ascontiguousarray(np.concatenate([xtp[2], xtp[3]], axis=1))
    ident = np.eye(128, dtype=ml_dtypes.bfloat16)

    tri = np.tril(np.ones((C, C), dtype=np.float32))  # [j, i] valid j<=i
    mask = np.ascontiguousarray(np.tile(tri.T, (1, CPT)))  # [j, i] 1 iff j<=i

    in_maps = []
    for core in range(NCORES):
        heads = [HPC * core + i for i in range(HPC)]
        # torch.chunk order in reference: q, v, k
        qcols = np.concatenate([np.arange(E * h, E * h + E) for h in heads])
        vcols = qcols + D
        kcols = qcols + 2 * D
        Wc = pack(
            np.concatenate([W[:, qcols], W[:, kcols], W[:, vcols]], axis=1).astype(
                ml_dtypes.bfloat16
            )
        )
        bq = b[qcols].reshape(128, 1)
        bk = b[kcols].reshape(128, 1)
        bv = np.broadcast_to(b[vcols], (128, HPC * E))
        mega = np.ascontiguousarray(np.concatenate([Wc, ident, xtp[0]], axis=1))
        cf = np.ascontiguousarray(
            np.concatenate([bq, bk, bv, mask], axis=1, dtype=np.float32)
        )
        in_maps.append(dict(mega=mega, cf=cf, xt1=xt1, xt23=xt23))
    return in_maps


_CACHE = {}


def kernel(x, W_qvk, b_qvk, head_num):
    assert int(np.asarray(head_num)) == H
    if "nc" not in _CACHE:
        _CACHE["nc"] = build_nc()
    nc = _CACHE["nc"]
    in_maps = host_inputs(x, W_qvk, b_qvk)
    from concourse.bass_utils import run_bass_kernel_spmd

    res = run_bass_kernel_spmd(
        nc,
        in_maps,
        core_ids=list(range(NCORES)),
        trace=bool(int(os.environ.get("KERNEL_TRACE", "0"))),
    )
    _CACHE["last_result"] = res
    out = np.concatenate([r["out"] for r in res.results], axis=1)
    return out.reshape(B, N, D).astype(np.float32)
